# revision 1
# baseline (speedup 1.0000x reference)
"""Bass/Trainium2 kernel for nn_BayesianBertSelfAttention (B=2,S=1024,HID=768,NH=12,HD=64).

Sharding: 24 (batch, head) pairs over 8 cores -> core c handles batch c//4,
heads {3k, 3k+1, 3k+2} with k = c%4.

Per-core device algorithm (transposed-scores layout, scoresT[r, l]):
  phase P: q/k projections as 4 column-packed matmul groups (qT/kT [64, S]
           bf16), v projections directly in [r, d] layout (bf16, with a ones
           column producing softmax row sums via the context matmul).
  phase R (pipelined into S): relative-position table R'[l, c] = q . E_rev
           band per 128-row l-tile (bf16 matmul), copied bf16 to a DRAM
           scratch with row pitch 1152.
  phase S: per head: skewed bf16 read of R' gives bias[l, r] tiles (the
           Music-Transformer skew as a strided DRAM access pattern); PE
           transposes them to [r, l] in a bf16 psum; one DVE add fuses
           psum scores + bias -> bf16 SBUF. Dual softmax: ACT exp(scale=1/8)
           over a fused [128, 2048] global|local pair (local pre-multiplied
           by SM^T on GPSIMD). Unnormalized bf16 probs feed two context
           matmuls; the ones-column of v gives row sums in row 64.
  phase F: PE transposes ctxT back to [l, d], DVE normalizes (reciprocal of
           row sums), blends the two branches with selector weights, DMA out.

Host: packs weights/binds layouts, converts to bf16, reassembles [2,1024,768].
"""

import sys

sys.path.insert(0, "/opt/trn_rl_repo")

import numpy as np
import ml_dtypes
from contextlib import ExitStack

import concourse.bass as bass
import concourse.bacc as bacc
import concourse.tile as tile
from concourse import mybir
from concourse.bass_utils import run_bass_kernel_spmd
from concourse.masks import make_identity

B, S, HID, NH, HD = 2, 1024, 768, 12, 64
MAXP = 1024
NCORES = 8
HPC = 3            # heads per core
LTN = S // 128     # 8 l-tiles
BAND = 1151        # skew band width per 128-row l-tile
PITCH = 1152       # skew scratch row pitch
NE = 2 * MAXP - 1  # 2047

BF16 = mybir.dt.bfloat16
F32 = mybir.dt.float32
FP8 = mybir.dt.float8e4
COPY = mybir.ActivationFunctionType.Copy
EXP = mybir.ActivationFunctionType.Exp

NPBF16 = ml_dtypes.bfloat16

_programs = {}


def _bcast(ap, dim_count, insert_at):
    """Insert a step-0 broadcast dim of size dim_count at free-dim position."""
    new = list(ap.ap)
    new.insert(insert_at, [0, dim_count])
    return bass.AP(tensor=ap.tensor, offset=ap.offset, ap=new)


def build_program(n_cc=6, use_m=False):
    """n_cc: number of 128-row contraction chunks for projections (6 normally,
    7 when biases are nonzero and folded in as an extra ones row)."""
    nc = bacc.Bacc(None)
    CH = n_cc * 128

    hidT = nc.dram_tensor("hidT", [CH, S], BF16, kind="ExternalInput")
    wg = nc.dram_tensor("wg", [4, CH, 128], BF16, kind="ExternalInput")
    wv = nc.dram_tensor("wv", [CH, HPC * HD], BF16, kind="ExternalInput")
    embT2 = nc.dram_tensor("embT2", [128, NE], BF16, kind="ExternalInput")
    smT = nc.dram_tensor("smT", [S, S], BF16, kind="ExternalInput")
    selw = nc.dram_tensor("selw", [S, 2], F32, kind="ExternalInput")
    if use_m:
        mvec = nc.dram_tensor("mvec", [S, 2], F32, kind="ExternalInput")  # [m, 8m]
    outp = nc.dram_tensor("out", [S, HPC * HD], F32, kind="ExternalOutput")
    skews = [nc.dram_tensor(f"skew{h}", [LTN * 128 * PITCH], FP8)
             for h in range(HPC)]

    # (q_h, k_h) SBUF partition offsets per head; G-groups: 0=[q0|q1], 1=[k0|k1],
    # 2=[q2|-], 3=[k2|-]
    QG = [(0, 0), (0, 64), (2, 0)]   # (group, partition offset) for q
    KG = [(1, 0), (1, 64), (3, 0)]

    with tile.TileContext(nc) as tc, ExitStack() as ctx:
        singles = ctx.enter_context(tc.tile_pool(name="singles", bufs=1))

        hid_sb = singles.tile([128, n_cc, S], BF16)
        wg_sb = singles.tile([128, 4, n_cc, 128], BF16)
        wv_sb = singles.tile([128, n_cc, HPC * HD], BF16)
        emb_sb = singles.tile([128, NE], BF16)
        smT_sb = singles.tile([128, 8, S], BF16)
        selw_sb = singles.tile([128, 8, 2], F32)
        hid_v = hidT.rearrange("(cc p) l -> p cc l", p=128)
        wg_v = wg.rearrange("g (cc p) d -> p g cc d", p=128)
        nc.sync.dma_start(out=wg_sb[:, 0], in_=wg_v[:, 0])
        for cc in range(n_cc):
            nc.sync.dma_start(out=hid_sb[:, cc], in_=hid_v[:, cc])
        for g in range(1, 4):
            nc.sync.dma_start(out=wg_sb[:, g], in_=wg_v[:, g])
        nc.sync.dma_start(out=emb_sb, in_=embT2[:, :])
        nc.sync.dma_start(out=wv_sb, in_=wv.rearrange("(cc p) d -> p cc d", p=128))
        if use_m:
            m_sb = singles.tile([128, 8, 2], F32)
            nc.sync.dma_start(out=m_sb, in_=mvec.rearrange("(rs p) w -> p rs w", p=128))

        identB = singles.tile([128, 128], BF16)
        make_identity(nc, identB)
        identb = singles.tile([65, 65], BF16)
        make_identity(nc, identb)

        qkT_sb = singles.tile([128, 4, S], BF16)     # G-group projection outputs
        v4t_sb = singles.tile([128, 8, HPC * 65], BF16)  # v_aug per r-subtile
        cg_sb = singles.tile([65, HPC, S], BF16)     # unnormalized ctxT, global
        cl_sb = singles.tile([65, HPC, S], BF16)     # local

        # ---- phase P || R: projections + positional bands, one psum scope ----
        with tc.tile_pool(name="ps_pr", bufs=2, space="PSUM") as ps_pr, \
             tc.tile_pool(name="ps_tl", bufs=2, space="PSUM") as ps_tl, \
             tc.tile_pool(name="ps_pt", bufs=1, space="PSUM") as ps_pt, \
             tc.tile_pool(name="rsp", bufs=5) as rsp:

            def emit_G(g):
                mg = 128 if g < 2 else 64
                pt = ps_pt.tile([128, S], F32, tag="pt")
                for n in range(2):
                    for cc in range(n_cc):
                        nc.tensor.matmul(
                            pt[:mg, n * 512:(n + 1) * 512],
                            lhsT=wg_sb[:, g, cc, :mg],
                            rhs=hid_sb[:, cc, n * 512:(n + 1) * 512],
                            start=(cc == 0), stop=(cc == n_cc - 1),
                        )
                nc.scalar.activation(qkT_sb[:mg, g, :], pt[:mg], COPY)

            def emit_R(h):
                g, po = QG[h]
                qb = qkT_sb[po:po + 64, g, :]
                for lt in range(LTN):
                    pr = ps_pr.tile([128, S], F32, tag="big")
                    prt = ps_tl.tile([128, BAND - S], F32, tag="tail")
                    e0 = 896 - lt * 128
                    qbl = qb[:, lt * 128:(lt + 1) * 128]
                    for n0, n1 in ((0, 512), (512, 1024)):
                        nc.tensor.matmul(
                            pr[:, n0:n1],
                            lhsT=qbl,
                            rhs=emb_sb[po:po + 64, e0 + n0:e0 + n1],
                            start=True, stop=True,
                        )
                    nc.tensor.matmul(
                        prt, lhsT=qbl,
                        rhs=emb_sb[po:po + 64, e0 + S:e0 + BAND],
                        start=True, stop=True,
                    )
                    rt = rsp.tile([128, BAND], FP8, tag="rt")
                    if lt % 2 == 0:
                        nc.scalar.activation(rt[:, 0:S], pr, COPY)
                        nc.scalar.activation(rt[:, S:BAND], prt, COPY)
                    else:
                        nc.vector.tensor_copy(rt[:, 0:S], pr)
                        nc.vector.tensor_copy(rt[:, S:BAND], prt)
                    wview = skews[h][lt * 128 * PITCH:(lt + 1) * 128 * PITCH] \
                        .rearrange("(p c) -> p c", c=PITCH)[:, 0:BAND]
                    nc.sync.dma_start(out=wview, in_=rt)

            emit_G(0)
            emit_R(0)
            emit_G(1)
            emit_R(1)
            emit_G(2)
            emit_R(2)
            emit_G(3)
            nc.vector.memset(
                v4t_sb.rearrange("p rs (h x) -> p rs h x", x=65)[:, :, :, 64], 1.0
            )
            for rs in range(8):
                pv = ps_pt.tile([128, HPC * HD], F32, tag="pt")
                for cc in range(n_cc):
                    nc.tensor.matmul(
                        pv,
                        lhsT=hid_sb[:, cc, rs * 128:(rs + 1) * 128],
                        rhs=wv_sb[:, cc, :],
                        start=(cc == 0), stop=(cc == n_cc - 1),
                    )
                nc.vector.tensor_copy(
                    v4t_sb[:, rs, :].rearrange("p (h x) -> p h x", x=65)[:, :, 0:64],
                    pv.rearrange("p (h d) -> p h d", d=64),
                )

        smT_v = smT.rearrange("(rs p) l -> p rs l", p=128)
        for rs in range(8):
            nc.sync.dma_start(out=smT_sb[:, rs], in_=smT_v[:, rs])
        nc.sync.dma_start(out=selw_sb, in_=selw.rearrange("(lc p) w -> p lc w", p=128))

        # ---- phase S: scores + dual softmax + context (+ v projection) ----
        with tc.tile_pool(name="ps_s", bufs=1, space="PSUM") as ps_s, \
             tc.tile_pool(name="ps_bt", bufs=2, space="PSUM") as ps_bt, \
             tc.tile_pool(name="ps_cg", bufs=1, space="PSUM") as ps_cg, \
             tc.tile_pool(name="ps_cl", bufs=1, space="PSUM") as ps_cl, \
             tc.tile_pool(name="wk", bufs=6) as wk, \
             tc.tile_pool(name="bskp", bufs=2) as bskp:

            def emit_S(h):
                bsk8 = bskp.tile([128, LTN, S], FP8, tag="bsk8")
                bsk = bskp.tile([128, LTN, S], BF16, tag="bsk")
                for lt in range(LTN):
                    base = lt * 128 * PITCH
                    rview = skews[h][base + 127:base + 127 + 128 * BAND] \
                        .rearrange("(p c) -> p c", c=BAND)[:, 0:S]
                    nc.sync.dma_start(out=bsk8[:, lt, :], in_=rview)
                    nc.gpsimd.tensor_copy(bsk[:, lt, :], bsk8[:, lt, :])
                qg, qpo = QG[h]
                kg, kpo = KG[h]
                qf = qkT_sb[qpo:qpo + 64, qg, :]
                kf = qkT_sb[kpo:kpo + 64, kg, :]
                cg = ps_cg.tile([65, S], F32, tag="cg")
                cl = ps_cl.tile([65, S], F32, tag="cl")
                for rs in range(8):
                    # positional bias, transposed to [r, l] in a bf16 psum
                    bt = ps_bt.tile([128, S], BF16, tag="bt")
                    for lt in range(LTN):
                        nc.tensor.matmul(
                            bt[:, lt * 128:(lt + 1) * 128],
                            lhsT=bsk[:, lt, rs * 128:(rs + 1) * 128],
                            rhs=identB,
                            is_transpose=True, start=True, stop=True,
                        )
                    # raw scores (q.k), f32 psum
                    st = ps_s.tile([128, S], F32, tag="st")
                    for n in range(2):
                        nc.tensor.matmul(
                            st[:, n * 512:(n + 1) * 512],
                            lhsT=kf[:, rs * 128:(rs + 1) * 128],
                            rhs=qf[:, n * 512:(n + 1) * 512],
                            start=True, stop=True,
                        )
                    if use_m:
                        nc.vector.tensor_scalar_add(st, st, m_sb[:, rs, 1:2])
                    btc = wk.tile([128, S], BF16, tag="btc")
                    if rs % 3 == 2:
                        nc.scalar.activation(btc, bt, COPY)
                    else:
                        nc.vector.tensor_copy(btc, bt)
                    # sgtl = [scores+bias | (scores+bias)*smT], bf16
                    sgtl = wk.tile([128, 2 * S], BF16, tag="sgtl")
                    nc.vector.tensor_add(sgtl[:, 0:S], st, btc)
                    nc.gpsimd.tensor_mul(sgtl[:, S:2 * S], sgtl[:, 0:S],
                                         smT_sb[:, rs, :])
                    pgl = wk.tile([128, 2 * S], BF16, tag="pgl")
                    if use_m:
                        nc.scalar.activation(pgl[:, 0:S], sgtl[:, 0:S], EXP,
                                             scale=0.125)
                        nc.scalar.activation(pgl[:, S:2 * S], sgtl[:, S:2 * S],
                                             EXP, scale=0.125,
                                             bias=m_sb[:, rs, 0:1])
                    else:
                        nc.scalar.activation(pgl, sgtl, EXP, scale=0.125)
                    va = v4t_sb[:, rs, h * 65:(h + 1) * 65]
                    for n in range(2):
                        nc.tensor.matmul(
                            cg[:, n * 512:(n + 1) * 512],
                            lhsT=va, rhs=pgl[:, n * 512:(n + 1) * 512],
                            start=(rs == 0), stop=(rs == 7),
                        )
                        nc.tensor.matmul(
                            cl[:, n * 512:(n + 1) * 512],
                            lhsT=va, rhs=pgl[:, S + n * 512:S + (n + 1) * 512],
                            start=(rs == 0), stop=(rs == 7),
                        )
                nc.scalar.activation(cg_sb[:, h, :], cg, COPY)
                nc.vector.tensor_copy(cl_sb[:, h, :], cl)

            for h in range(HPC):
                emit_S(h)

        # ---- phase F: transpose back, normalize, blend, store ----
        with tc.tile_pool(name="ps_f", bufs=2, space="PSUM") as ps_f, \
             tc.tile_pool(name="fin", bufs=3) as fin:
            for lc in range(LTN):
                pf = ps_f.tile([128, 6 * 66], BF16, tag="pf")
                for h in range(HPC):
                    for br, csb in enumerate((cg_sb, cl_sb)):
                        x = h * 2 + br
                        nc.tensor.matmul(
                            pf[:, x * 66:x * 66 + 65],
                            lhsT=csb[:, h, lc * 128:(lc + 1) * 128],
                            rhs=identb,
                            is_transpose=True, start=True, stop=True,
                        )
                pfv = pf.rearrange("p (x c) -> p x c", c=66)
                rsum = fin.tile([128, 6], F32, tag="rsum")
                nc.vector.reciprocal(rsum, pfv[:, :, 64])
                w = fin.tile([128, 6], F32, tag="w")
                selv = selw_sb[:, lc, :]  # [128, 2]; col0=(1-sel) for g, col1=sel
                nc.vector.tensor_mul(
                    w.rearrange("p (h b) -> p h b", b=2),
                    rsum.rearrange("p (h b) -> p h b", b=2),
                    _bcast(selv, 3, 1),
                )
                tmp = fin.tile([128, 6, 64], F32, tag="tmp")
                nc.vector.tensor_mul(tmp, pfv[:, :, 0:64], _bcast(w, 64, 2))
                osb = fin.tile([128, HPC * HD], F32, tag="osb")
                tv = tmp.rearrange("p (h b) d -> p h b d", b=2)
                nc.vector.tensor_add(
                    osb.rearrange("p (h d) -> p h d", d=64),
                    tv[:, :, 0, :], tv[:, :, 1, :],
                )
                nc.sync.dma_start(out=outp[lc * 128:(lc + 1) * 128, :], in_=osb)

    nc.compile()
    return nc


def _get_program(n_cc, use_m):
    key = (n_cc, use_m)
    if key not in _programs:
        _programs[key] = build_program(n_cc, use_m)
    return _programs[key]


def kernel(hidden_states, attention_mask, scaled_attention_mask, selector_outputs,
           Wq, bq, Wk, bk, Wv, bv, dist_emb):
    hidden_states = np.asarray(hidden_states, np.float32)
    attention_mask = np.asarray(attention_mask, np.float32)
    scaled_attention_mask = np.asarray(scaled_attention_mask, np.float32)
    selector_outputs = np.asarray(selector_outputs, np.float32)
    Wq, Wk, Wv = (np.asarray(x, np.float32) for x in (Wq, Wk, Wv))
    bq, bk, bv = (np.asarray(x, np.float32) for x in (bq, bk, bv))
    dist_emb = np.asarray(dist_emb, np.float32)

    use_bias = bool(np.any(bq) or np.any(bk) or np.any(bv))
    use_m = bool(np.any(attention_mask))
    n_cc = 7 if use_bias else 6
    CH = n_cc * 128
    nc = _get_program(n_cc, use_m)

    smT = np.ascontiguousarray(scaled_attention_mask[0, 0].T).astype(NPBF16)
    e_rev_t = dist_emb[::-1].T.astype(NPBF16)
    embT2 = np.ascontiguousarray(np.concatenate([e_rev_t, e_rev_t], axis=0))

    in_maps = []
    for core in range(NCORES):
        b = core // 4
        k4 = core % 4
        heads = [3 * k4, 3 * k4 + 1, 3 * k4 + 2]

        hidT = hidden_states[b].T  # [768, S]
        if use_bias:
            hidT = np.concatenate(
                [hidT, np.ones((1, S), np.float32),
                 np.zeros((CH - HID - 1, S), np.float32)], axis=0)
        hidT_bf = np.ascontiguousarray(hidT).astype(NPBF16)

        def wcols(W, bvec, h):
            c = W[:, h * HD:(h + 1) * HD]
            if use_bias:
                c = np.concatenate(
                    [c, bvec[None, h * HD:(h + 1) * HD],
                     np.zeros((CH - HID - 1, HD), np.float32)], axis=0)
            return c

        q0, q1, q2 = (wcols(Wq, bq, h) for h in heads)
        k0, k1, k2 = (wcols(Wk, bk, h) for h in heads)
        z = np.zeros_like(q2)
        wg_np = np.stack([
            np.concatenate([q0, q1], axis=1),
            np.concatenate([k0, k1], axis=1),
            np.concatenate([q2, z], axis=1),
            np.concatenate([k2, z], axis=1),
        ]).astype(NPBF16)
        wv_np = np.concatenate(
            [wcols(Wv, bv, h) for h in heads], axis=1).astype(NPBF16)

        sel = selector_outputs[b, 0, :, 0]
        selw_np = np.stack([1.0 - sel, sel], axis=1).astype(np.float32)

        m = {
            "hidT": hidT_bf,
            "wg": wg_np,
            "wv": np.ascontiguousarray(wv_np),
            "embT2": embT2,
            "smT": smT,
            "selw": np.ascontiguousarray(selw_np),
        }
        if use_m:
            mv = attention_mask[b, 0, 0]
            m["mvec"] = np.ascontiguousarray(
                np.stack([mv, 8.0 * mv], axis=1).astype(np.float32))
        in_maps.append(m)

    res = run_bass_kernel_spmd(nc, in_maps, list(range(NCORES)))

    out = np.empty((B, S, HID), np.float32)
    for core in range(NCORES):
        b = core // 4
        k4 = core % 4
        out[b, :, 192 * k4:192 * (k4 + 1)] = res.results[core]["out"]
    return out



# revision 36
# speedup vs baseline: 1.0738x; 1.0738x over previous
"""Bass/Trainium2 kernel for nn_BayesianBertSelfAttention (B=2,S=1024,HID=768,NH=12,HD=64).

Sharding: 24 (batch, head) pairs over 8 cores -> core c handles batch c//4,
heads {3k, 3k+1, 3k+2} with k = c%4.

Per-core algorithm (scores transposed, st[r, l], fp8 DoubleRow matmuls):
  phase P: q/k projections as fp8 DoubleRow matmuls over cc-chunk PAIRS
           (host interleaves hidT/weights as [128, pair, 2, .]); psum
           copies emit q8z/k8z [*, 2, S] fp8 with a zero t=1 slot.  v
           projection stays bf16 (precision-critical), with a ones column
           for softmax row sums.
  phase R: relative-position band R'[l, c] = q . (64*E_rev) per 128-row
           l-tile via DR matmuls (rhs = emb8 with a step-0 broadcast pair
           dim); psum -> fp8 rt tiles -> DRAM skew scratch (pitch 1152).
  skew read: one strided DMA per half-head lands the skewed band in
           PARTITION-FOLDED layout bsk8f [64, 2, lt, S] (rows p and p+64
           of each l-tile share a partition, split along the pair dim).
  phase S: per (head, r-subtile): scores via fp8 DR matmuls (lhsT = k8z
           with zero slot, rhs = broadcast q8) PLUS 8 bias "transposes" as
           DR matmuls against a folded identity I2/64 -- all accumulate
           into ONE f32 psum st[r, l].  ACT exp(0.125*st) -> pg (global);
           Pool computes (st [+8m]) * smT -> loc; ACT exp -> pl (local).
           Context matmuls stay bf16; ones-column of v gives row sums.
  phase F: PE transposes ctxT back to [l, d], DVE normalizes + blends with
           selector weights, one batched DMA out.

Host packs/interleaves weights, scales fp8 tensors by 64 where needed
(scaled back via activation scale=1/64 or the I2/64 identity).
"""

import sys

sys.path.insert(0, "/opt/trn_rl_repo")

import numpy as np
import ml_dtypes
from contextlib import ExitStack

import concourse.bass as bass
import concourse.bacc as bacc
import concourse.tile as tile
from concourse import mybir
from concourse.bass_utils import run_bass_kernel_spmd

B, S, HID, NH, HD = 2, 1024, 768, 12, 64
MAXP = 1024
NCORES = 8
HPC = 3            # heads per core
LTN = S // 128     # 8 l-tiles
BAND = 1151        # skew band width per 128-row l-tile
PITCH = 1152       # skew scratch row pitch
NE = 2 * MAXP - 1  # 2047

BF16 = mybir.dt.bfloat16
F32 = mybir.dt.float32
FP8 = mybir.dt.float8e4
COPY = mybir.ActivationFunctionType.Copy
EXP = mybir.ActivationFunctionType.Exp
DR = mybir.MatmulPerfMode.DoubleRow

NPBF16 = ml_dtypes.bfloat16
NPFP8 = ml_dtypes.float8_e4m3fn

_programs = {}


def _bcast(ap, dim_count, insert_at):
    """Insert a step-0 broadcast dim of size dim_count at free-dim position."""
    new = list(ap.ap)
    new.insert(insert_at, [0, dim_count])
    return bass.AP(tensor=ap.tensor, offset=ap.offset, ap=new)


def _mkap(base_ap, offset, dims):
    """Raw AP on base_ap's tensor: offset + [[stride, num], ...]."""
    return bass.AP(tensor=base_ap.tensor, offset=offset, ap=[list(d) for d in dims])


def build_program(n_cc=6, use_m=False):
    """n_cc: contraction chunks for projections (6 normally, 7 with biases
    folded as an extra ones row)."""
    nc = bacc.Bacc(None)
    CH = n_cc * 128
    NCP = (n_cc + 1) // 2  # cc-pairs for DR band-q projection (zero-padded)

    hidT = nc.dram_tensor("hidT", [128, n_cc, S], BF16, kind="ExternalInput")
    hid8 = nc.dram_tensor("hid8", [128, NCP, 2, S], FP8, kind="ExternalInput")
    wg = nc.dram_tensor("wg", [128, 4, n_cc, 128], BF16, kind="ExternalInput")
    wg8q = nc.dram_tensor("wg8q", [128, 2, NCP, 2, 128], FP8, kind="ExternalInput")
    wv = nc.dram_tensor("wv", [128, n_cc, HPC * HD], BF16, kind="ExternalInput")
    emb8 = nc.dram_tensor("emb8", [128, NE], FP8, kind="ExternalInput")
    smT = nc.dram_tensor("smT", [128, 8, S], FP8, kind="ExternalInput")
    selw = nc.dram_tensor("selw", [128, 8, 2], F32, kind="ExternalInput")
    if use_m:
        mvec = nc.dram_tensor("mvec", [128, 8, 2], F32, kind="ExternalInput")
    outp = nc.dram_tensor("out", [S, HPC * HD], F32, kind="ExternalOutput")
    skews = [nc.dram_tensor(f"skew{h}", [LTN * 128 * PITCH], FP8)
             for h in range(HPC)]

    with tile.TileContext(nc) as tc, ExitStack() as ctx:
        singles = ctx.enter_context(tc.tile_pool(name="singles", bufs=1))

        hid8_sb = singles.tile([128, NCP, 2, S], FP8)
        wg_sb = singles.tile([128, 4, n_cc, 128], BF16)
        wg8q_sb = singles.tile([128, 2, NCP, 2, 128], FP8)
        hid_sb = singles.tile([128, n_cc, S], BF16)
        wv_sb = singles.tile([128, n_cc, HPC * HD], BF16)
        emb8_sb = singles.tile([128, NE], FP8)
        smT_sb = singles.tile([128, 8, S], FP8)
        selw_sb = singles.tile([128, 8, 2], F32)

        q8z = singles.tile([128, 2, S], FP8)     # band q, heads 0/1 (+zeros)
        qb8 = singles.tile([64, 2, S], FP8)      # band q, head 2 (+zeros)
        qkT_sb = singles.tile([128, 4, S], BF16)  # bf16 q/k for scores
        v4t_sb = singles.tile([128, 8, HPC * 65], BF16)
        cg_sb = singles.tile([65, HPC, S], BF16)
        cl_sb = singles.tile([65, HPC, S], BF16)
        i2a = singles.tile([128, 2, 128], FP8)   # [I/64 | 0]
        i2b = singles.tile([128, 2, 128], FP8)   # [0 | I/64]
        identb = singles.tile([65, 65], BF16)

        # ---- input DMAs ----
        nc.sync.dma_start(out=wg8q_sb, in_=wg8q[:, :, :, :, :])
        nc.sync.dma_start(out=hid8_sb, in_=hid8[:, :, :, :])
        nc.sync.dma_start(out=emb8_sb, in_=emb8[:, :])
        nc.sync.dma_start(out=smT_sb[:, 0:4, :], in_=smT[:, 0:4, :])
        nc.gpsimd.dma_start(out=wv_sb, in_=wv[:, :, :])
        nc.gpsimd.dma_start(out=hid_sb[:, 0:2, :], in_=hidT[:, 0:2, :])
        if use_m:
            m_sb = singles.tile([128, 8, 2], F32)

        # ---- constants ----
        for tens, tv in ((i2a, 0), (i2b, 1)):
            tf = tens.rearrange("p t l -> p (t l)")
            nc.gpsimd.memset(tf, 0.0)
            nc.gpsimd.affine_select(
                out=tf, in_=tf, compare_op=mybir.AluOpType.not_equal,
                fill=1.0 / 64.0, base=tv * 128, pattern=[[-1, 256]],
                channel_multiplier=1,
            )
        nc.gpsimd.memset(identb, 0.0)
        nc.gpsimd.affine_select(
            out=identb, in_=identb, compare_op=mybir.AluOpType.not_equal,
            fill=1.0, base=0, pattern=[[-1, 65]], channel_multiplier=1,
        )
        nc.gpsimd.memset(q8z[:, 1, :], 0.0)
        nc.gpsimd.memset(qb8[:, 1, :], 0.0)
        nc.vector.memset(
            v4t_sb.rearrange("p rs (h x) -> p rs h x", x=65)[:, :, :, 64], 1.0
        )

        # bf16 G-groups in qkT_sb: 0=[q0|q1], 1=[k0|k1], 2=[q2|-], 3=[k2|-]
        QG = [(0, 0), (0, 64), (2, 0)]
        KG = [(1, 0), (1, 64), (3, 0)]
        QPO = [0, 64, 0]                 # band fp8 q partition offsets

        def qtens(h):
            return q8z if h < 2 else qb8

        def skew_wview(h, lt0, nlt):
            return _mkap(skews[h][0:1], lt0 * 128 * PITCH,
                         [[PITCH, 128], [128 * PITCH, nlt], [1, BAND]])

        def skew_rview(h, t):
            return _mkap(skews[h][0:1], 127 + t * 128 * PITCH,
                         [[BAND, 128], [2 * 128 * PITCH, 4], [1, S]])

        def copy_tile(eng, dst, srcp):
            if eng is nc.scalar:
                eng.activation(dst, srcp, COPY)
            else:
                eng.tensor_copy(dst, srcp)

        # per-tile copy engines: h0 balanced 3-way, h1 leans DVE (ACT busy)
        CPENG = {0: "ADADADAD", 1: "DADADDAD"}

        # SBUF pools that span phases
        wk = ctx.enter_context(tc.tile_pool(name="wk", bufs=3))
        rsp = ctx.enter_context(tc.tile_pool(name="rsp", bufs=2))
        bskp = ctx.enter_context(tc.tile_pool(name="bskp", bufs=2))

        bsk = [None] * HPC

        def read_skew(h):
            t = bskp.tile([128, 4, 2, S], FP8, tag="bsk")
            for tv in range(2):
                nc.sync.dma_start(out=t[:, :, tv, :], in_=skew_rview(h, tv))
            return t

        # ======== scope 1a: band-q fp8 projection ========
        with tc.tile_pool(name="ps_pa", bufs=1, space="PSUM") as ps_pa:

            def emit_q8(g8):
                mg = 128 if g8 == 0 else 64
                pt = ps_pa.tile([128, S], F32, tag="pt")
                for n in range(2):
                    for cp in range(NCP):
                        nc.tensor.matmul(
                            pt[:mg, n * 512:(n + 1) * 512],
                            lhsT=wg8q_sb[:, g8, cp, :, :mg],
                            rhs=hid8_sb[:, cp, :, n * 512:(n + 1) * 512],
                            start=(cp == 0), stop=(cp == NCP - 1),
                            perf_mode=DR,
                        )
                dst = q8z[:, 0, :] if g8 == 0 else qb8[:, 0, :]
                nc.scalar.activation(dst, pt[:mg], COPY, scale=1.0 / 64)

            emit_q8(0)
            emit_q8(1)
            nc.scalar.dma_start(out=wg_sb, in_=wg[:, :, :, :])
            nc.scalar.dma_start(out=selw_sb, in_=selw[:, :, :])
            if use_m:
                nc.scalar.dma_start(out=m_sb, in_=mvec[:, :, :])

        # ======== scope 1b: bands h0, h1 (pr triple-buffered) ========
        with tc.tile_pool(name="ps_pr", bufs=3, space="PSUM") as ps_pr, \
             tc.tile_pool(name="ps_tl", bufs=2, space="PSUM") as ps_tl:

            def band_mm(h, lt, pr, prt):
                po = QPO[h]
                qt = qtens(h)
                e0 = 896 - lt * 128
                lhs = qt[po:po + 64, :, lt * 128:(lt + 1) * 128]
                for n0, n1 in ((0, 512), (512, 1024)):
                    nc.tensor.matmul(
                        pr[:, n0:n1], lhsT=lhs,
                        rhs=_bcast(emb8_sb[po:po + 64, e0 + n0:e0 + n1], 2, 1),
                        start=True, stop=True, perf_mode=DR,
                    )
                nc.tensor.matmul(
                    prt, lhsT=lhs,
                    rhs=_bcast(emb8_sb[po:po + 64, e0 + S:e0 + BAND], 2, 1),
                    start=True, stop=True, perf_mode=DR,
                )

            def emit_R(h, hooks=None):
                rt8 = rsp.tile([128, 8, PITCH], FP8, tag="rt8")
                for lt in range(LTN):
                    pr = ps_pr.tile([128, S], F32, tag="big")
                    prt = ps_tl.tile([128, BAND - S], F32, tag="tail")
                    band_mm(h, lt, pr, prt)
                    eng = {"A": nc.scalar, "P": nc.gpsimd,
                           "D": nc.vector}[CPENG[h][lt]]
                    copy_tile(eng, rt8[:, lt, 0:S], pr)
                    copy_tile(eng, rt8[:, lt, S:BAND], prt)
                    if lt % 2 == 1:
                        nc.sync.dma_start(
                            out=skew_wview(h, lt - 1, 2),
                            in_=rt8[:, lt - 1:lt + 1, 0:BAND])
                    if hooks and lt in hooks:
                        hooks[lt]()
                return rt8

            def read_skew(h, second_q=None):
                t = bskp.tile([128, 4, 2, S], FP8, tag="bsk")
                nc.sync.dma_start(out=t[:, :, 0, :], in_=skew_rview(h, 0))
                q2_ = second_q or nc.sync
                q2_.dma_start(out=t[:, :, 1, :], in_=skew_rview(h, 1))
                return t

            h0_hooks = {
                2: lambda: nc.gpsimd.dma_start(out=hid_sb[:, 2:4, :],
                                               in_=hidT[:, 2:4, :]),
                5: lambda: nc.gpsimd.dma_start(out=hid_sb[:, 4:n_cc, :],
                                               in_=hidT[:, 4:n_cc, :]),
            }
            emit_R(0, h0_hooks)
            bsk[0] = read_skew(0, second_q=nc.scalar)
            emit_R(1)
            bsk[1] = read_skew(1)
            nc.sync.dma_start(out=smT_sb[:, 4:8, :], in_=smT[:, 4:8, :])

        # ======== scope 2: bf16 projections g0 (q01), g1 (k01) ========
        def emit_G(g, pool):
            mg = 128 if g < 2 else 64
            pt = pool.tile([128, S], F32, tag="ptb")
            for n in range(2):
                for cc in range(n_cc):
                    nc.tensor.matmul(
                        pt[:mg, n * 512:(n + 1) * 512],
                        lhsT=wg_sb[:, g, cc, :mg],
                        rhs=hid_sb[:, cc, n * 512:(n + 1) * 512],
                        start=(cc == 0), stop=(cc == n_cc - 1),
                    )
            nc.scalar.activation(qkT_sb[:mg, g, :], pt[:mg], COPY)

        with tc.tile_pool(name="ps_pb", bufs=2, space="PSUM") as ps_pb:
            emit_G(0, ps_pb)
            emit_G(1, ps_pb)

        # ======== phase S (+ trailing v-proj / band-h2 / g2 g3) ========
        def emit_v_rs(rs, pool):
            pv = pool.tile([128, HPC * HD], F32, tag="pv")
            for cc in range(n_cc):
                nc.tensor.matmul(
                    pv,
                    lhsT=hid_sb[:, cc, rs * 128:(rs + 1) * 128],
                    rhs=wv_sb[:, cc, :],
                    start=(cc == 0), stop=(cc == n_cc - 1),
                )
            nc.vector.tensor_copy(
                v4t_sb[:, rs, :].rearrange("p (h x) -> p h x", x=65)[:, :, 0:64],
                pv.rearrange("p (h d) -> p h d", d=64),
            )

        with tc.tile_pool(name="ps_st3", bufs=3, space="PSUM") as ps_st3, \
             tc.tile_pool(name="ps_cg", bufs=1, space="PSUM") as ps_cg, \
             tc.tile_pool(name="ps_cl", bufs=1, space="PSUM") as ps_cl:

            trail = ExitStack()
            pv_pool = trail.enter_context(
                tc.tile_pool(name="ps_pv", bufs=1, space="PSUM"))
            for rs in range(3):
                emit_v_rs(rs, pv_pool)

            fillers = []

            def emit_S(h):
                qg, qpo = QG[h]
                kg, kpo = KG[h]
                qf = qkT_sb[qpo:qpo + 64, qg, :]
                kf = qkT_sb[kpo:kpo + 64, kg, :]
                bh = bsk[h]
                cg = ps_cg.tile([65, S], F32, tag="cg")
                cl = ps_cl.tile([65, S], F32, tag="cl")
                pgls = [None] * 8

                def emit_ctx(rs):
                    va = v4t_sb[:, rs, h * 65:(h + 1) * 65]
                    pgl = pgls[rs]
                    for n in range(2):
                        nc.tensor.matmul(
                            cg[:, n * 512:(n + 1) * 512],
                            lhsT=va, rhs=pgl[:, 2 * n, :],
                            start=(rs == 0), stop=(rs == 7),
                        )
                        nc.tensor.matmul(
                            cl[:, n * 512:(n + 1) * 512],
                            lhsT=va, rhs=pgl[:, 2 * n + 1, :],
                            start=(rs == 0), stop=(rs == 7),
                        )

                for rs in range(8):
                    sth = [ps_st3.tile([128, 512], F32, tag="sth", name="stl"),
                           ps_st3.tile([128, 512], F32, tag="sth", name="str")]
                    for n in range(2):
                        nc.tensor.matmul(
                            sth[n],
                            lhsT=kf[:, rs * 128:(rs + 1) * 128],
                            rhs=qf[:, n * 512:(n + 1) * 512],
                            start=True, stop=False,
                            skip_group_check=True,
                        )
                    for lt in range(LTN):
                        nc.tensor.matmul(
                            sth[lt // 4][:, (lt % 4) * 128:(lt % 4 + 1) * 128],
                            lhsT=bh[:, lt // 2, :, rs * 128:(rs + 1) * 128],
                            rhs=(i2a if lt % 2 == 0 else i2b),
                            start=False, stop=True, perf_mode=DR,
                            skip_group_check=True,
                        )
                    pgl = wk.tile([128, 4, 512], BF16, tag="pgl")
                    sgtl = wk.tile([128, 4, 512], BF16, tag="sgtl")
                    for n in range(2):
                        if use_m:
                            nc.vector.tensor_scalar_add(
                                sgtl[:, 2 * n, :], sth[n], m_sb[:, rs, 1:2])
                        else:
                            nc.vector.tensor_copy(sgtl[:, 2 * n, :], sth[n])
                        nc.gpsimd.tensor_mul(
                            sgtl[:, 2 * n + 1, :], sgtl[:, 2 * n, :],
                            smT_sb[:, rs, n * 512:(n + 1) * 512])
                    sgtlf = sgtl.rearrange("p t l -> p (t l)")
                    pglf = pgl.rearrange("p t l -> p (t l)")
                    if use_m:
                        gv = pgl.rearrange("p t l -> p t l")[:, 0::2, :]
                        lv = pgl.rearrange("p t l -> p t l")[:, 1::2, :]
                        sgv = sgtl.rearrange("p t l -> p t l")[:, 0::2, :]
                        slv = sgtl.rearrange("p t l -> p t l")[:, 1::2, :]
                        nc.scalar.activation(gv, sgv, EXP, scale=0.125)
                        nc.scalar.activation(lv, slv, EXP, scale=0.125,
                                             bias=m_sb[:, rs, 0:1])
                    else:
                        nc.scalar.activation(pglf, sgtlf, EXP, scale=0.125)
                    pgls[rs] = pgl
                    if rs > 0:
                        emit_ctx(rs - 1)
                    for _ in range(3):
                        if fillers:
                            fillers.pop(0)()
                emit_ctx(7)
                if h == 2:
                    nc.vector.tensor_copy(cg_sb[:, h, 0:512], cg[:, 0:512])
                    nc.scalar.activation(cg_sb[:, h, 512:1024],
                                         cg[:, 512:1024], COPY)
                    nc.vector.tensor_copy(cl_sb[:, h, 0:512], cl[:, 0:512])
                    nc.scalar.activation(cl_sb[:, h, 512:1024],
                                         cl[:, 512:1024], COPY)
                else:
                    nc.vector.tensor_copy(cg_sb[:, h, :], cg)
                    nc.vector.tensor_copy(cl_sb[:, h, :], cl)

            # fillers for S(0): v-proj first (ctx deps), then band-h2 + skew
            order0 = [("v", 3), ("v", 4), ("v", 5), ("v", 6), ("v", 7),
                      ("b", 0), ("b", 1), ("b", 2), ("b", 3), ("b", 4),
                      ("b", 5), ("b", 6), ("b", 7)]

            bnd_state = {}

            def open_bnd():
                trail.pop_all().close()
                bnd_state["pool"] = trail.enter_context(
                    tc.tile_pool(name="ps_bn", bufs=1, space="PSUM"))
                bnd_state["rt8"] = rsp.tile([128, 8, PITCH], FP8, tag="rt8", name="rt8b")

            def band_h2_chunk(lt, ck):
                if "pool" not in bnd_state:
                    open_bnd()
                pool = bnd_state["pool"]
                rt8 = bnd_state["rt8"]
                po = QPO[2]
                e0 = 896 - lt * 128
                lhs = qb8[:, :, lt * 128:(lt + 1) * 128]
                pr = pool.tile([128, 512], F32, tag="b1", name="prc")
                if ck < 2:
                    nc.tensor.matmul(
                        pr, lhsT=lhs,
                        rhs=_bcast(emb8_sb[po:po + 64,
                                           e0 + ck * 512:e0 + (ck + 1) * 512],
                                   2, 1),
                        start=True, stop=True, perf_mode=DR)
                    nc.vector.tensor_copy(
                        rt8[:, lt, ck * 512:(ck + 1) * 512], pr)
                else:
                    nc.tensor.matmul(
                        pr[:, 0:BAND - S], lhsT=lhs,
                        rhs=_bcast(emb8_sb[po:po + 64, e0 + S:e0 + BAND], 2, 1),
                        start=True, stop=True, perf_mode=DR)
                    nc.vector.tensor_copy(rt8[:, lt, S:BAND],
                                          pr[:, 0:BAND - S])
                    if lt % 2 == 1:
                        nc.sync.dma_start(
                            out=skew_wview(2, lt - 1, 2),
                            in_=rt8[:, lt - 1:lt + 1, 0:BAND])
                    if lt == 7:
                        bsk[2] = read_skew(2)

            for kind, idx in order0:
                if kind == "b":
                    for ck in range(3):
                        fillers.append(
                            lambda lt=idx, ck=ck: band_h2_chunk(lt, ck))
                else:
                    fillers.append(lambda rs=idx: emit_v_rs(rs, pv_pool))

            emit_S(0)

            # fillers for S(1): g2, g3 (bf16, 2-cc accumulation chunks)
            def open_g23():
                trail.pop_all().close()
                bnd_state["gp"] = trail.enter_context(
                    tc.tile_pool(name="ps_g23", bufs=1, space="PSUM"))

            def g23_piece(g, n, k):
                if "gp" not in bnd_state:
                    open_g23()
                pool = bnd_state["gp"]
                if k == 0:
                    bnd_state["gh"] = pool.tile([64, 512], F32, tag="gh",
                                                name="gh")
                pt = bnd_state["gh"]
                ccs = range(2 * k, min(2 * k + 2, n_cc))
                for cc in ccs:
                    nc.tensor.matmul(
                        pt,
                        lhsT=wg_sb[:, g, cc, :64],
                        rhs=hid_sb[:, cc, n * 512:(n + 1) * 512],
                        start=(cc == 0), stop=(cc == n_cc - 1),
                    )
                if 2 * k + 2 >= n_cc:
                    nc.vector.tensor_copy(
                        qkT_sb[:64, g, n * 512:(n + 1) * 512], pt)

            nk = (n_cc + 1) // 2
            for g in (2, 3):
                for n in range(2):
                    for k in range(nk):
                        fillers.append(
                            lambda g=g, n=n, k=k: g23_piece(g, n, k))

            emit_S(1)
            trail.pop_all().close()
            emit_S(2)
            assert not fillers

        # ======== phase F ========
        osb8 = singles.tile([128, 8, HPC * HD], F32)
        with tc.tile_pool(name="ps_f", bufs=4, space="PSUM") as ps_f, \
             tc.tile_pool(name="fin", bufs=6) as fin:
            for lc in range(LTN):
                pf = ps_f.tile([128, 6 * 66], BF16, tag="pf")
                for h in range(HPC):
                    for br, csb in enumerate((cg_sb, cl_sb)):
                        x = h * 2 + br
                        nc.tensor.matmul(
                            pf[:, x * 66:x * 66 + 65],
                            lhsT=csb[:, h, lc * 128:(lc + 1) * 128],
                            rhs=identb,
                            is_transpose=True, start=True, stop=True,
                        )
                pfv = pf.rearrange("p (x c) -> p x c", c=66)
                rsum = fin.tile([128, 6], F32, tag="rsum")
                nc.vector.reciprocal(rsum, pfv[:, :, 64])
                w = fin.tile([128, 6], F32, tag="w")
                selv = selw_sb[:, lc, :]
                nc.gpsimd.tensor_mul(
                    w.rearrange("p (h b) -> p h b", b=2),
                    rsum.rearrange("p (h b) -> p h b", b=2),
                    _bcast(selv, 3, 1),
                )
                tmp = fin.tile([128, 6, 64], F32, tag="tmp")
                nc.vector.tensor_mul(tmp, pfv[:, :, 0:64], _bcast(w, 64, 2))
                tv = tmp.rearrange("p (h b) d -> p h b d", b=2)
                nc.vector.tensor_add(
                    osb8[:, lc, :].rearrange("p (h d) -> p h d", d=64),
                    tv[:, :, 0, :], tv[:, :, 1, :],
                )
                if lc % 4 == 3:
                    nc.sync.dma_start(
                        out=_mkap(outp[0:1, 0:1],
                                  (lc - 3) * 128 * HPC * HD,
                                  [[HPC * HD, 128], [128 * HPC * HD, 4],
                                   [1, HPC * HD]]),
                        in_=osb8[:, lc - 3:lc + 1, :],
                    )

    nc.compile()
    return nc


def _get_program(n_cc, use_m):
    key = (n_cc, use_m)
    if key not in _programs:
        _programs[key] = build_program(n_cc, use_m)
    return _programs[key]


def _pack_core_inputs(core, hidden_states, attention_mask, scaled_attention_mask,
                      selector_outputs, Wq, bq, Wk, bk, Wv, bv, dist_emb,
                      use_bias, use_m, smT, emb8, n_cc):
    CH = n_cc * 128
    NCP = (n_cc + 1) // 2
    b = core // 4
    k4 = core % 4
    heads = [3 * k4, 3 * k4 + 1, 3 * k4 + 2]

    hidT = hidden_states[b].T  # [768, S]
    if use_bias:
        hidT = np.concatenate(
            [hidT, np.ones((1, S), np.float32),
             np.zeros((CH - HID - 1, S), np.float32)], axis=0)
    # hidT (bf16, v-proj): [128, n_cc, S] partition-major
    hidT_bf = np.ascontiguousarray(
        hidT.reshape(n_cc, 128, S).transpose(1, 0, 2)).astype(NPBF16)

    # hid8: [128, NCP, 2, S]; chunk index cc = 2*cp + t; zero-pad odd chunk
    hid_pad = hidT
    if 2 * NCP * 128 > CH:
        hid_pad = np.concatenate(
            [hidT, np.zeros((2 * NCP * 128 - CH, S), np.float32)], axis=0)
    hid8_np = np.ascontiguousarray(
        hid_pad.reshape(NCP, 2, 128, S).transpose(2, 0, 1, 3)).astype(NPFP8)

    def wcols(W, bvec, h):
        c = W[:, h * HD:(h + 1) * HD]
        if use_bias:
            c = np.concatenate(
                [c, bvec[None, h * HD:(h + 1) * HD],
                 np.zeros((CH - HID - 1, HD), np.float32)], axis=0)
        return c

    q0, q1, q2 = (wcols(Wq, bq, h) for h in heads)
    k0, k1, k2 = (wcols(Wk, bk, h) for h in heads)
    z = np.zeros_like(q2)
    groups = [
        np.concatenate([q0, q1], axis=1),
        np.concatenate([k0, k1], axis=1),
        np.concatenate([q2, z], axis=1),
        np.concatenate([k2, z], axis=1),
    ]
    # wg: bf16 [128, 4, n_cc, 128] partition-major
    wg_np = np.stack(
        [W.reshape(n_cc, 128, 128).transpose(1, 0, 2) for W in groups],
        axis=1).astype(NPBF16)

    # wg8q: fp8 q-groups only [128, 2, NCP, 2, 128] = 64*W, cc = 2*cp + t
    qgroups = [groups[0], groups[2]]
    wg8q_np = np.zeros((128, 2, NCP, 2, 128), np.float32)
    for g, W in enumerate(qgroups):
        Wp = W
        if 2 * NCP * 128 > CH:
            Wp = np.concatenate(
                [W, np.zeros((2 * NCP * 128 - CH, 128), np.float32)], axis=0)
        wg8q_np[:, g] = 64.0 * Wp.reshape(NCP, 2, 128, 128).transpose(2, 0, 1, 3)
    wg8q_np = wg8q_np.astype(NPFP8)

    # wv: [128, n_cc, 192] partition-major bf16
    wv_full = np.concatenate([wcols(Wv, bv, h) for h in heads], axis=1)
    wv_np = np.ascontiguousarray(
        wv_full.reshape(n_cc, 128, HPC * HD).transpose(1, 0, 2)).astype(NPBF16)

    sel = selector_outputs[b, 0, :, 0]
    selw_full = np.stack([1.0 - sel, sel], axis=1).astype(np.float32)  # [S, 2]
    selw_np = np.ascontiguousarray(
        selw_full.reshape(8, 128, 2).transpose(1, 0, 2))

    m = {
        "hidT": hidT_bf,
        "hid8": hid8_np,
        "wg": np.ascontiguousarray(wg_np),
        "wg8q": np.ascontiguousarray(wg8q_np),
        "wv": wv_np,
        "emb8": emb8,
        "smT": smT,
        "selw": selw_np,
    }
    if use_m:
        mv = attention_mask[b, 0, 0]
        mvec_full = np.stack([mv, 8.0 * mv], axis=1).astype(np.float32)
        m["mvec"] = np.ascontiguousarray(
            mvec_full.reshape(8, 128, 2).transpose(1, 0, 2))
    return m


def kernel(hidden_states, attention_mask, scaled_attention_mask, selector_outputs,
           Wq, bq, Wk, bk, Wv, bv, dist_emb):
    hidden_states = np.asarray(hidden_states, np.float32)
    attention_mask = np.asarray(attention_mask, np.float32)
    scaled_attention_mask = np.asarray(scaled_attention_mask, np.float32)
    selector_outputs = np.asarray(selector_outputs, np.float32)
    Wq, Wk, Wv = (np.asarray(x, np.float32) for x in (Wq, Wk, Wv))
    bq, bk, bv = (np.asarray(x, np.float32) for x in (bq, bk, bv))
    dist_emb = np.asarray(dist_emb, np.float32)

    use_bias = bool(np.any(bq) or np.any(bk) or np.any(bv))
    use_m = bool(np.any(attention_mask))
    n_cc = 7 if use_bias else 6
    nc = _get_program(n_cc, use_m)

    smT_t = scaled_attention_mask[0, 0].T  # [S(r), S(l)]
    smT = np.ascontiguousarray(
        smT_t.reshape(8, 128, S).transpose(1, 0, 2)).astype(NPFP8)
    e_rev_t = dist_emb[::-1].T  # [64, 2047]
    emb8 = np.ascontiguousarray(
        np.concatenate([e_rev_t, e_rev_t], axis=0) * 64.0).astype(NPFP8)

    in_maps = [
        _pack_core_inputs(core, hidden_states, attention_mask,
                          scaled_attention_mask, selector_outputs,
                          Wq, bq, Wk, bk, Wv, bv, dist_emb,
                          use_bias, use_m, smT, emb8, n_cc)
        for core in range(NCORES)
    ]

    res = run_bass_kernel_spmd(nc, in_maps, list(range(NCORES)))

    out = np.empty((B, S, HID), np.float32)
    for core in range(NCORES):
        b = core // 4
        k4 = core % 4
        out[b, :, 192 * k4:192 * (k4 + 1)] = res.results[core]["out"]
    return out
